# revision 8
# baseline (speedup 1.0000x reference)
"""Trainium2 Bass kernel v3 for nn_CrossAttention_78305843740743.

Baseline's proven Chebyshev-in-x / powers-in-y expansion of tanh(ql+kl)
(rel err ~1.1e-2), re-hosted in a restructured pipeline:
  - host pre-transposes + f16-casts key/query/weights: no PE transposes,
    no transpose evacs, ~half the input DMA bytes of the original.
  - q/k projections f16 (PE), evacs read PSUM directly.
  - k-side powers ladder split ACT Square / DVE tensor_tensor.
  - f16 output, cast to f32 on host.

Sharding: 8 cores = (batch 2) x (q-chunk 2 of 128) x (k-half 2 of 1024).
"""

import math
from contextlib import ExitStack

import numpy as np

import concourse.bacc as bacc
import concourse.bass as bass
import concourse.tile as tile
from concourse import mybir
from concourse.bass_utils import run_bass_kernel_spmd

F32 = mybir.dt.float32
F16 = mybir.dt.float16
AF = mybir.ActivationFunctionType
OP = mybir.AluOpType
P = 128

BSZ, NUM_Q, NUM_K = 2, 256, 2048
D_Q, D_K, D_ATT = 512, 512, 128
N_CORES = 8
Q_SHARD = 128
K_SHARD = 1024
KO = D_K // P
NG = 2
R = 10

S_K = 4.778631080638971
S_Q = 4.791558761070814
# C[i,j]: coefficient of T_i(ql/S_Q) for (kl/S_K)^j (baseline fit)
C_MIX = [
    [(1, 1.211185804e+00), (3, -3.707771977e-01), (5, 1.695732166e-01), (7, -1.031641916e-01), (11, -5.894060152e-02)],
    [(8, 1.222745770e+00), (2, -1.219663442e+00), (4, 9.928071873e-01), (0, 6.551535978e-01), (6, -2.569379808e-01)],
    [(11, 2.988445899e+00), (7, 2.698290098e+00), (5, -2.217486950e+00), (3, 1.762865864e+00), (2, -5.641671060e-04)],
    [(8, -1.608020989e+01), (4, -2.980990406e+00), (6, -1.606740325e+00), (2, 7.579222721e-01), (12, -1.131876017e-01)],
    [(11, -2.527959028e+01), (7, -1.191019264e+01), (5, 4.441181540e+00), (2, 9.204531641e-03)],
    [(8, 6.072535114e+01), (6, 1.592122417e+01), (4, 2.102677772e+00), (12, -2.693594617e-01)],
    [(11, 7.569054867e+01), (7, 1.653126803e+01), (5, -1.091209568e+00), (2, -1.128175098e-02)],
    [(8, -8.635367936e+01), (6, -3.007109208e+01), (12, 1.830723534e+00)],
    [(11, -9.282531554e+01), (7, -6.519513302e+00), (9, -1.857507791e+00)],
    [(8, 4.110390690e+01), (6, 1.662484277e+01), (12, -1.565889661e+00)],
    [(11, 3.954030205e+01), (9, 2.002297269e+00)],
]
# power m -> (a, b, engine) with t^m = t^a * t^b
CHAIN = {2: (1, 1, 'act'), 3: (2, 1, 'dve'), 4: (2, 2, 'act'),
         5: (3, 2, 'dve'), 6: (3, 3, 'act'), 7: (4, 3, 'dve'),
         8: (4, 4, 'act'), 9: (5, 4, 'dve'), 10: (5, 5, 'act')}

_CACHED = {}


def _build_bass(n_iters: int = 1) -> bass.Bass:
    nc = bacc.Bacc("TRN2", target_bir_lowering=False, debug=False,
                   num_devices=N_CORES)

    wk_d = nc.dram_tensor("wk", [P, 2, KO, D_ATT], F16, kind="ExternalInput").ap()
    wq_d = nc.dram_tensor("wq", [P, 2, KO, D_ATT], F16, kind="ExternalInput").ap()
    # vec cols: 0 blk/S_K, 1 bbk, 2 blq, 3 bbq, 4 v_att, 5 b_att
    vec_d = nc.dram_tensor("vec", [P, 8], F32, kind="ExternalInput").ap()
    qT_d = nc.dram_tensor("qT", [P, KO, Q_SHARD], F16, kind="ExternalInput").ap()
    keyT_d = nc.dram_tensor("keyT", [P, KO, K_SHARD], F16, kind="ExternalInput").ap()
    out_d = nc.dram_tensor("out", [Q_SHARD, K_SHARD], F16, kind="ExternalOutput").ap()

    with tile.TileContext(nc) as tc, ExitStack() as ctx:
        if n_iters > 1:
            ctx.enter_context(tc.For_i(0, n_iters, 1,
                                       hint_engines=(mybir.EngineType.PE,)))
        sgl = ctx.enter_context(tc.tile_pool(name="sgl", bufs=1))
        pj_psum = ctx.enter_context(tc.tile_pool(name="pj_psum", bufs=2, space="PSUM"))
        kl_psum = ctx.enter_context(tc.tile_pool(name="kl_psum", bufs=2, space="PSUM"))
        kb_psum = ctx.enter_context(tc.tile_pool(name="kb_psum", bufs=2, space="PSUM"))
        out_psum = ctx.enter_context(tc.tile_pool(name="out_psum", bufs=NG, space="PSUM"))

        keyT = sgl.tile([P, KO, K_SHARD], F16, tag="keyT")
        nc.sync.dma_start(out=keyT[:, 0:1, :], in_=keyT_d[:, 0:1, :])
        nc.scalar.dma_start(out=keyT[:, 1:2, :], in_=keyT_d[:, 1:2, :])
        nc.sync.dma_start(out=keyT[:, 2:3, :], in_=keyT_d[:, 2:3, :])
        nc.scalar.dma_start(out=keyT[:, 3:4, :], in_=keyT_d[:, 3:4, :])
        wq = sgl.tile([P, 2, KO, D_ATT], F16, tag="wq")
        nc.gpsimd.dma_start(out=wq, in_=wq_d)
        qT = sgl.tile([P, KO, Q_SHARD], F16, tag="qT")
        nc.gpsimd.dma_start(out=qT, in_=qT_d)
        wk = sgl.tile([P, 2, KO, D_ATT], F16, tag="wk")
        nc.gpsimd.dma_start(out=wk, in_=wk_d)
        vec = sgl.tile([P, 8], F32, tag="vec")
        nc.gpsimd.dma_start(out=vec, in_=vec_d)

        blkS, bbk, blq, bbq = (vec[:, 0:1], vec[:, 1:2], vec[:, 2:3], vec[:, 3:4])
        vT, batt = vec[:, 4:5], vec[:, 5:6]

        # ---- q projections first ----
        pql = pj_psum.tile([P, P], F32, tag="pj", name="pql")
        for c in range(KO):
            nc.tensor.matmul(pql, wq[:, 0, c, :], qT[:, c, :],
                             start=(c == 0), stop=(c == KO - 1))
        pqb = pj_psum.tile([P, P], F32, tag="pj", name="pqb")
        for c in range(KO):
            nc.tensor.matmul(pqb, wq[:, 1, c, :], qT[:, c, :],
                             start=(c == 0), stop=(c == KO - 1))

        # ---- k projections ----
        pkl = [kl_psum.tile([P, 512], F32, tag="kl", name=f"pkl{g}") for g in range(NG)]
        pkb = [kb_psum.tile([P, 512], F32, tag="kb", name=f"pkb{g}") for g in range(NG)]
        for c in range(KO):
            for g in range(NG):
                nc.tensor.matmul(pkl[g], wk[:, 0, c, :],
                                 keyT[:, c, g * 512:(g + 1) * 512],
                                 start=(c == 0), stop=(c == KO - 1))
        for c in range(KO):
            for g in range(NG):
                nc.tensor.matmul(pkb[g], wk[:, 1, c, :],
                                 keyT[:, c, g * 512:(g + 1) * 512],
                                 start=(c == 0), stop=(c == KO - 1))

        # ---- t powers ladder ----
        tp = {1: sgl.tile([P, K_SHARD], F16, tag="t1", name="t1")}
        for g in range(NG):
            nc.scalar.activation(tp[1][:, g * 512:(g + 1) * 512], pkl[g],
                                 AF.Identity, bias=blkS, scale=1.0 / S_K)
        for m in range(2, R + 1):
            tp[m] = sgl.tile([P, K_SHARD], F16, tag=f"tp{m}", name=f"tp{m}")
        for m in range(2, R + 1):
            a, b, eng = CHAIN[m]
            if eng == 'act':
                nc.scalar.activation(tp[m], tp[a], AF.Square)
            else:
                nc.vector.tensor_tensor(out=tp[m], in0=tp[a], in1=tp[b],
                                        op=OP.mult)

        # kb evac on ACT
        kbT = sgl.tile([P, K_SHARD], F16, tag="kbT")
        for g in range(NG):
            nc.scalar.activation(kbT[:, g * 512:(g + 1) * 512], pkb[g],
                                 AF.Identity, bias=bbk, scale=1.0)

        # ---- q side: u, Chebyshev tiles, mixing (baseline scheme) ----
        u = sgl.tile([P, P], F16, tag="u")
        nc.vector.tensor_scalar(out=u, in0=pql, scalar1=blq,
                                scalar2=1.0 / S_Q, op0=OP.add, op1=OP.mult)
        qbT = sgl.tile([P, P], F16, tag="qbT")
        nc.vector.tensor_scalar(out=qbT, in0=pqb, scalar1=bbq,
                                scalar2=1.0 / math.sqrt(D_ATT),
                                op0=OP.add, op1=OP.mult)

        needed = set()
        for terms in C_MIX:
            for i, _ in terms:
                needed.add(i)
        req = set(needed) - {0, 1}
        for i in sorted(req, reverse=True):
            req.update({x for x in ((i // 2), (i // 2 + 1 if i % 2 else i // 2))
                        if x > 1})
        T = {1: u}
        for i in sorted(req):
            a, b = (i // 2, i // 2) if i % 2 == 0 else (i // 2, i // 2 + 1)
            tmp = sgl.tile([P, P], F16, tag=f"ttmp{i}")
            nc.vector.tensor_tensor(out=tmp, in0=T[a], in1=T[b], op=OP.mult)
            Ti = sgl.tile([P, P], F16, tag=f"Tch{i}")
            if i % 2 == 0:
                nc.vector.tensor_scalar(out=Ti, in0=tmp, scalar1=2.0,
                                        scalar2=1.0, op0=OP.mult,
                                        op1=OP.subtract)
            else:
                nc.vector.scalar_tensor_tensor(out=Ti, in0=tmp, scalar=2.0,
                                               in1=u, op0=OP.mult,
                                               op1=OP.subtract)
            T[i] = Ti
        ones32 = sgl.tile([P, P], F16, tag="ones32")
        nc.vector.memset(ones32, 1.0)
        T[0] = ones32

        slabs = [None] * (R + 1)
        for j in list(range(1, R + 1)) + [0]:
            terms = C_MIX[j]
            acc = sgl.tile([P, P], F16, tag=f"acc{j}", name=f"acc{j}")
            i0, c0 = terms[0]
            nc.vector.tensor_scalar(out=acc, in0=T[i0], scalar1=float(c0),
                                    scalar2=0.0, op0=OP.mult, op1=OP.add)
            for i, cij in terms[1:]:
                nc.vector.scalar_tensor_tensor(out=acc, in0=T[i],
                                               scalar=float(cij), in1=acc,
                                               op0=OP.mult, op1=OP.add)
            slab = sgl.tile([P, P], F16, tag=f"slab{j}", name=f"slab{j}")
            nc.vector.tensor_scalar(out=slab, in0=acc, scalar1=vT,
                                    scalar2=0.0, op0=OP.mult, op1=OP.add)
            slabs[j] = slab

        # j=0 rank-1 -> qbias (+ b_att)
        ones16 = sgl.tile([P, 1], F16, tag="ones16")
        nc.vector.memset(ones16, 1.0)
        s0ps = pj_psum.tile([P, 1], F32, tag="pj", name="s0ps")
        nc.tensor.matmul(s0ps, slabs[0], ones16, start=True, stop=True)
        qbias = sgl.tile([P, 1], F32, tag="qbias")
        nc.vector.tensor_scalar(out=qbias, in0=s0ps, scalar1=batt,
                                scalar2=None, op0=OP.add)

        # ---- accumulation: 10 powers + bilinear ----
        po = [out_psum.tile([P, 512], F32, tag="po", name=f"po{g}")
              for g in range(NG)]
        for g in range(NG):
            nc.tensor.matmul(po[g], qbT, kbT[:, g * 512:(g + 1) * 512],
                             start=True, stop=False, skip_group_check=True)
        for j in range(1, R + 1):
            for g in range(NG):
                nc.tensor.matmul(po[g], slabs[j],
                                 tp[j][:, g * 512:(g + 1) * 512],
                                 start=False, stop=(j == R),
                                 skip_group_check=True)

        # ---- evac + store (f16) ----
        out_sb = sgl.tile([Q_SHARD, K_SHARD], F16, tag="out_sb")
        nc.vector.tensor_scalar(out=out_sb[:, 0:512], in0=po[0],
                                scalar1=qbias, scalar2=None, op0=OP.add)
        nc.sync.dma_start(out=out_d[:, 0:512], in_=out_sb[:, 0:512])
        nc.scalar.activation(out_sb[:, 512:1024], po[1],
                             AF.Identity, bias=qbias, scale=1.0)
        nc.gpsimd.dma_start(out=out_d[:, 512:1024], in_=out_sb[:, 512:1024])

    nc.compile()
    return nc


def _get_nc() -> bass.Bass:
    if "nc" not in _CACHED:
        _CACHED["nc"] = _build_bass()
    return _CACHED["nc"]


def make_in_maps(**inputs) -> list[dict[str, np.ndarray]]:
    f32 = lambda x: np.asarray(x, dtype=np.float32)
    query = f32(inputs["query"])
    key = f32(inputs["key"])
    pack = lambda w: np.ascontiguousarray(
        f32(w).reshape(KO, P, D_ATT).transpose(1, 0, 2)).astype(np.float16)
    wk = np.ascontiguousarray(np.stack(
        [pack(inputs["W_lk"]), pack(inputs["W_bk"])], axis=1))
    wq = np.ascontiguousarray(np.stack(
        [pack(inputs["W_lq"]), pack(inputs["W_bq"])], axis=1))

    vec = np.zeros((8, P), np.float32)
    vec[0] = f32(inputs["b_lk"]) / S_K
    vec[1] = f32(inputs["b_bk"])
    vec[2] = f32(inputs["b_lq"])
    vec[3] = f32(inputs["b_bq"])
    vec[4] = f32(inputs["v_att"])
    vec[5] = np.float32(np.asarray(inputs["b_att"], np.float32).reshape(()))
    vec = np.ascontiguousarray(vec.T)

    shared = {"wk": wk, "wq": wq, "vec": vec}
    in_maps = []
    for c in range(N_CORES):
        b, qc, kh = c // 4, (c // 2) % 2, c % 2
        qs = query[b, qc * Q_SHARD:(qc + 1) * Q_SHARD, :]
        ks = key[b, kh * K_SHARD:(kh + 1) * K_SHARD, :]
        qTa = np.ascontiguousarray(
            qs.T.reshape(KO, P, Q_SHARD).transpose(1, 0, 2)).astype(np.float16)
        keyTa = np.ascontiguousarray(
            ks.T.reshape(KO, P, K_SHARD).transpose(1, 0, 2)).astype(np.float16)
        in_maps.append({"qT": qTa, "keyT": keyTa, **shared})
    return in_maps


def assemble(results: list[dict[str, np.ndarray]]) -> np.ndarray:
    out = np.empty((BSZ, NUM_Q, NUM_K), np.float32)
    for c in range(N_CORES):
        b, qc, kh = c // 4, (c // 2) % 2, c % 2
        out[b, qc * Q_SHARD:(qc + 1) * Q_SHARD,
            kh * K_SHARD:(kh + 1) * K_SHARD] = results[c]["out"].astype(np.float32)
    return out


def kernel(**inputs) -> np.ndarray:
    nc = _get_nc()
    in_maps = make_in_maps(**inputs)
    res = run_bass_kernel_spmd(nc, in_maps, list(range(N_CORES)))
    return assemble(res.results)
